# revision 18
# baseline (speedup 1.0000x reference)
"""GQA attention (14 q heads, 2 kv heads, RoPE, causal) on 8 Trainium2 cores.

Sharding: core = (batch b, group g); each core owns the 512 query positions
{16*(g+4a)+i : a<32, i<16} of batch b (16-wide blocks strided by 64), which
gives every core an identical causal work shape (uniform SPMD program) while
balancing the causal triangle. Outputs are disjoint per core; no collectives.

Per-core q-chunk t (128 packed rows) attends exactly k-tiles 0..4t+3 for every
core; the partially-masked diagonal window is the last 4 k-tiles, with the
bias pattern host-supplied per core. Scores are computed twice on PE: once as
s[q,k] (softmax + attn_weights output, denominator via ACT accum_out) and once
as sT[k,q] for A@V (denominator double-checked via a ones column in V).
"""

import numpy as np

B, S, H = 2, 2048, 896
NH, NKV, HD = 14, 2, 64
N_CORES = 8
NEG = -1e9

_CACHE = {}


def _positions(g):
    blocks = np.arange(32) * 4 + g
    return (blocks[:, None] * 16 + np.arange(16)[None, :]).reshape(-1)


def _build():
    import sys
    for p in ("/opt/trn_rl_repo", "/root/.axon_site/_ro/trn_rl_repo"):
        if p not in sys.path:
            sys.path.append(p)
    from contextlib import ExitStack
    import concourse.tile as tile
    from concourse import bacc, mybir

    dt = mybir.dt
    F32 = dt.float32
    F32R = dt.float32r
    EXP = mybir.ActivationFunctionType.Exp
    IDENT = mybir.ActivationFunctionType.Identity
    MULT = mybir.AluOpType.mult
    MULT_OP0 = mybir.AluOpType.mult
    ADD = mybir.AluOpType.add

    nc = bacc.Bacc("TRN2", target_bir_lowering=False, debug=False,
                   num_devices=N_CORES)

    def din(name, shape, dtype=F32R):
        return nc.dram_tensor(name, shape, dtype, kind="ExternalInput").ap()

    hiddenT = din("hiddenT", [H, S])
    hQ = din("hQ", [H, 512])
    WqT = din("WqT", [H, H])
    WkT = din("WkT", [H, 128])
    WvT = din("WvT", [H, 128])
    WoT = din("WoT", [H, H])
    bqT = din("bqT", [128, 7], F32)
    bkT = din("bkT", [128, 1], F32)
    bvT = din("bvT", [128, 1], F32)
    cosq = din("cosq", [128, 512])
    ssinq = din("ssinq", [128, 512])
    cosk = din("cosk", [128, S])
    ssink = din("ssink", [128, S])
    maskS = din("maskS", [128, 512], F32)
    maskT4 = din("maskT4", [128, 512], F32)
    # 0/1 variants (multiplicative masking on SBUF, off the PSUM path)
    ident = din("ident", [128, 128])

    w_out = nc.dram_tensor("w_out", [NH, 512, S], F32, kind="ExternalOutput").ap()
    o_out = nc.dram_tensor("o_out", [512, H], F32, kind="ExternalOutput").ap()

    with tile.TileContext(nc) as tc:
        with ExitStack() as es:
            consts = es.enter_context(tc.tile_pool(name="consts", bufs=1))
            persist = es.enter_context(tc.tile_pool(name="persist", bufs=1))
            ps_scores = es.enter_context(
                tc.tile_pool(name="scores", bufs=3, space="PSUM"))
            small = es.enter_context(tc.tile_pool(name="small", bufs=4))
            ph1sp = es.enter_context(tc.tile_pool(name="ph1sp", bufs=1))

            # ---- constants (persist whole kernel) ----
            maskS01_sb = consts.tile([128, 512], F32)
            nc.scalar.dma_start(maskS01_sb[:], maskS)
            maskT01_sb = consts.tile([128, 512], F32)
            nc.scalar.dma_start(maskT01_sb[:], maskT4)
            id_sb = consts.tile([128, 128], F32R)
            nc.scalar.dma_start(id_sb[:], ident)
            id32_sb = consts.tile([128, 128], F32)
            nc.scalar.dma_start(id32_sb[:], ident.bitcast(F32))
            bk_sb = consts.tile([128, 1], F32)
            nc.scalar.dma_start(bk_sb[:], bkT)
            bv_sb = consts.tile([128, 1], F32)
            nc.scalar.dma_start(bv_sb[:], bvT)
            bq_sb = consts.tile([128, 7], F32)
            nc.scalar.dma_start(bq_sb[:], bqT)

            # persistent phase-2 tensors (allocated as they are produced)
            qR = [persist.tile([128, 512], F32R, tag=f"qR{j}", name=f"qR{j}")
                  for j in range(7)]
            kt = ph1sp.tile([128, S], F32R)
            ktr = ph1sp.tile([128, S], F32R)
            vt = ph1sp.tile([128, S], F32R)

            # ---- phase 1a: projections (hidden + streamed weights) ----
            with tc.tile_pool(name="ph1a", bufs=1) as ph1a, \
                 tc.tile_pool(name="wload", bufs=2) as wload, \
                 tc.tile_pool(name="qtmp", bufs=2) as qtmp:
                hq = []
                for ic in range(7):
                    h_sb = ph1a.tile([128, 512], F32R, tag=f"hq{ic}", name=f"hq{ic}")
                    nc.scalar.dma_start(h_sb[:], hQ[128 * ic:128 * (ic + 1), :])
                    hq.append(h_sb)
                hid = []
                for ic in range(7):
                    h_sb = ph1a.tile([128, S], F32R, tag=f"hid{ic}", name=f"hid{ic}")
                    nc.sync.dma_start(h_sb[:], hiddenT[128 * ic:128 * (ic + 1), :])
                    hid.append(h_sb)
                cq_sb = ph1a.tile([128, 512], F32R)
                nc.scalar.dma_start(cq_sb[:], cosq)
                sq_sb = ph1a.tile([128, 512], F32R)
                nc.scalar.dma_start(sq_sb[:], ssinq)

                # q projections, streamed per out-chunk pair; rotate-half
                # comes from partition-shifted copies of the biased q.
                for ocg in range(4):
                    noc = 2 if ocg < 3 else 1
                    cols = slice(256 * ocg, 256 * ocg + 128 * noc)
                    pq = ps_scores.tile([128, 1024], F32, tag="scores", name="pq")
                    for ic in range(7):
                        wq_i = wload.tile([128, 256], F32R, tag="wqi", name="wq_i")
                        nc.scalar.dma_start(wq_i[:, 0:128 * noc],
                                          WqT[128 * ic:128 * (ic + 1), cols])
                        for e in range(noc):
                            nc.tensor.matmul(pq[:, 512 * e:512 * e + 512],
                                             wq_i[:, 128 * e:128 * (e + 1)],
                                             hq[ic][:],
                                             start=(ic == 0), stop=(ic == 6))
                    for e in range(noc):
                        oc = 2 * ocg + e
                        qt = qtmp.tile([128, 512], F32R, tag="qt", name="qt")
                        nc.scalar.activation(qt[:], pq[:, 512 * e:512 * e + 512],
                                             IDENT, bias=bq_sb[:, oc:oc + 1])
                        qtr = qtmp.tile([128, 512], F32R, tag="qtr", name="qtr")
                        for hb in range(2):
                            nc.vector.tensor_copy(
                                qtr[64 * hb:64 * hb + 32, :],
                                qt[64 * hb + 32:64 * hb + 64, :])
                            nc.vector.tensor_copy(
                                qtr[64 * hb + 32:64 * hb + 64, :],
                                qt[64 * hb:64 * hb + 32, :])
                        nc.vector.tensor_tensor(qt[:], qt[:].bitcast(F32),
                                                cq_sb[:].bitcast(F32), MULT)
                        nc.vector.tensor_tensor(qtr[:], qtr[:].bitcast(F32),
                                                sq_sb[:].bitcast(F32), MULT)
                        nc.vector.tensor_tensor(qR[oc][:], qt[:].bitcast(F32),
                                                qtr[:].bitcast(F32), ADD)

                # k / v projections (weights tiny: fully resident)
                wkv = []
                for nm, wsrc in (("wk", WkT), ("wv", WvT)):
                    tiles = []
                    for ic in range(7):
                        w_t = ph1a.tile([128, 128], F32R, tag=f"{nm}{ic}",
                                        name=f"{nm}{ic}")
                        nc.scalar.dma_start(w_t[:], wsrc[128 * ic:128 * (ic + 1), :])
                        tiles.append(w_t)
                    wkv.append(tiles)
                for w in range(4):
                    sl = slice(512 * w, 512 * (w + 1))
                    pk = ps_scores.tile([128, 1024], F32, tag="scores", name="pk")
                    pv = ps_scores.tile([128, 1024], F32, tag="scores", name="pv")
                    for ic in range(7):
                        nc.tensor.matmul(pk[:, 0:512], wkv[0][ic][:], hid[ic][:, sl],
                                         start=(ic == 0), stop=(ic == 6))
                        nc.tensor.matmul(pv[:, 0:512], wkv[1][ic][:], hid[ic][:, sl],
                                         start=(ic == 0), stop=(ic == 6))
                    nc.scalar.activation(kt[:, sl], pk[:, 0:512], IDENT, bias=bk_sb[:])
                    nc.scalar.activation(vt[:, sl], pv[:, 0:512], IDENT, bias=bv_sb[:])
                for hb in range(2):
                    nc.vector.tensor_copy(ktr[64 * hb:64 * hb + 32, :],
                                          kt[64 * hb + 32:64 * hb + 64, :])
                    nc.vector.tensor_copy(ktr[64 * hb + 32:64 * hb + 64, :],
                                          kt[64 * hb:64 * hb + 32, :])

            # ---- phase 1b: RoPE on k, kv-duplicated layouts, v transpose ----
            # opened only now so phase-1a gets the SBUF headroom
            persist2 = es.enter_context(tc.tile_pool(name="persist2", bufs=1))
            K00 = persist2.tile([128, S], F32R)
            K01 = persist2.tile([128, S], F32R)
            K11 = persist2.tile([128, S], F32R)
            vsb = [persist2.tile([128, 128], F32R, tag=f"v{k}", name=f"v{k}")
                   for k in range(16)]
            with tc.tile_pool(name="ph1b", bufs=1) as ph1b:
                ck_sb = ph1b.tile([128, S], F32R)
                nc.sync.dma_start(ck_sb[:], cosk)
                sk_sb = ph1b.tile([128, S], F32R)
                nc.sync.dma_start(sk_sb[:], ssink)
                nc.vector.tensor_tensor(kt[:], kt[:].bitcast(F32),
                                        ck_sb[:].bitcast(F32), MULT)
                nc.vector.tensor_tensor(ktr[:], ktr[:].bitcast(F32),
                                        sk_sb[:].bitcast(F32), MULT)
                nc.vector.tensor_tensor(K01[:], kt[:].bitcast(F32),
                                        ktr[:].bitcast(F32), ADD)
                nc.sync.dma_start(K00[0:64, :], K01[0:64, :])
                nc.sync.dma_start(K00[64:128, :], K01[0:64, :])
                nc.sync.dma_start(K11[0:64, :], K01[64:128, :])
                nc.sync.dma_start(K11[64:128, :], K01[64:128, :])
                for k in range(16):
                    pvT = ps_scores.tile([128, 1024], F32R, tag="scores", name="pvT")
                    nc.tensor.transpose(pvT[:, 0:128], vt[:, 128 * k:128 * (k + 1)],
                                        id_sb[:])
                    nc.vector.tensor_copy(vsb[k][:], pvT[:, 0:128])

            # ---- phase 2: attention, head-pairs interleaved ----
            outT_sb = [persist2.tile([128, 512], F32R, tag=f"oT{j}",
                                     name=f"oT{j}") for j in range(7)]
            KJ = [K00, K00, K00, K01, K11, K11, K11]
            ph3 = es.enter_context(tc.tile_pool(name="ph3", bufs=1))
            wo_all = []
            for half in range(2):
                row = []
                for hc in range(7):
                    w_t = ph3.tile([128, 448], F32R, tag=f"wo{half}_{hc}",
                                   name=f"wo{half}_{hc}")
                    nc.scalar.dma_start(
                        w_t[:], WoT[128 * hc:128 * (hc + 1),
                                    448 * half:448 * (half + 1)])
                    row.append(w_t)
                wo_all.append(row)
            with tc.tile_pool(name="wstage", bufs=3) as wstage, \
                 tc.tile_pool(name="estage", bufs=4) as estage, \
                 tc.tile_pool(name="ps_av", bufs=1, space="PSUM") as ps_av:
                for j in range(7):
                    kjf = KJ[j]
                    # --- s-path, both heads interleaved ---
                    # chunk-t denominator in column 32t so the PE transpose
                    # lands it at a 32-aligned partition
                    den4s = [small.tile([128, 97], F32, tag=f"den4_{hh}",
                                        name=f"den4_{hh}", bufs=2)
                             for hh in range(2)]
                    oTs = [ps_av.tile([64, 512], F32, tag=f"oT{hh}",
                                      name=f"oT{hh}") for hh in range(2)]
                    for t in range(4):
                        span = 512 * (t + 1)
                        wsb = [wstage.tile([128, 2048], F32, tag=f"w{hh}",
                                           name=f"w{hh}") for hh in range(2)]
                        # non-diagonal windows 0..t-1 (pairs of 512), then the
                        # diagonal window t alone; masking happens on SBUF.
                        pts = [[], []]
                        for tp in range((t + 1) // 2):
                            nwin = min(2, t - 2 * tp)
                            for hh in range(2):
                                pts[hh].append(
                                    (ps_scores.tile([128, 1024], F32,
                                                    tag="scores", name="pt"),
                                     nwin))
                            for e in range(nwin):
                                tau = 2 * tp + e
                                for hh in range(2):
                                    pi = 64 * hh
                                    nc.tensor.matmul(
                                        pts[hh][tp][0][:, 512 * e:512 * (e + 1)],
                                        qR[j][pi:pi + 64, 128 * t:128 * (t + 1)],
                                        kjf[pi:pi + 64, 512 * tau:512 * (tau + 1)],
                                        start=True, stop=True)
                        pds = []
                        for hh in range(2):
                            pi = 64 * hh
                            pd = ps_scores.tile([128, 1024], F32, tag="scores",
                                                name="pd")
                            nc.tensor.matmul(
                                pd[:, 0:512],
                                qR[j][pi:pi + 64, 128 * t:128 * (t + 1)],
                                kjf[pi:pi + 64, 512 * t:512 * (t + 1)],
                                start=True, stop=True)
                            pds.append(pd)
                        for hh in range(2):
                            dcol = 32 * t
                            dparts = []
                            for tp, (pt, nwin) in enumerate(pts[hh]):
                                acc = small.tile([128, 1], F32, tag="dpart",
                                                 name="dpart")
                                nc.scalar.activation(
                                    wsb[hh][:, 1024 * tp:1024 * tp + 512 * nwin],
                                    pt[:, 0:512 * nwin], EXP, accum_out=acc)
                                dparts.append(acc)
                            # diagonal: exp then 0/1-mask-mult with accum
                            nc.scalar.activation(wsb[hh][:, 512 * t:512 * (t + 1)],
                                                 pds[hh][:, 0:512], EXP)
                            nc.vector.scalar_tensor_tensor(
                                wsb[hh][:, 512 * t:512 * (t + 1)],
                                wsb[hh][:, 512 * t:512 * (t + 1)],
                                1.0, maskS01_sb[:], MULT_OP0, MULT,
                                accum_out=den4s[hh][:, dcol:dcol + 1])
                            for acc in dparts:
                                nc.vector.tensor_add(
                                    den4s[hh][:, dcol:dcol + 1],
                                    den4s[hh][:, dcol:dcol + 1], acc)
                        for hh in range(2):
                            dcol = 32 * t
                            rec = small.tile([128, 1], F32, tag="rec", name="rec")
                            nc.vector.reciprocal(rec[:],
                                                 den4s[hh][:, dcol:dcol + 1])
                            nc.vector.tensor_scalar_mul(wsb[hh][:, 0:span],
                                                        wsb[hh][:, 0:span], rec[:])
                            nc.sync.dma_start(
                                w_out[2 * j + hh, 128 * t:128 * (t + 1), 0:span],
                                wsb[hh][:, 0:span])

                    for g4 in range(4):
                        nq = 512 - 128 * g4
                        for d2 in range(2):
                            sts = []
                            ets = []
                            for hh in range(2):
                                sts.append(ps_scores.tile(
                                    [128, 1024], F32, tag="scores", name="st"))
                                ets.append(estage.tile(
                                    [128, 1024], F32R, tag="et", name="et"))
                            for e in range(2):
                                kk = 4 * g4 + 2 * d2 + e
                                for hh in range(2):
                                    pi = 64 * hh
                                    nc.tensor.matmul(
                                        sts[hh][:, 512 * e:512 * e + nq],
                                        kjf[pi:pi + 64, 128 * kk:128 * (kk + 1)],
                                        qR[j][pi:pi + 64, 128 * g4:512],
                                        start=True, stop=True)
                            for hh in range(2):
                                m0 = (4 * g4 + 2 * d2) % 4
                                in3 = sts[hh][:].rearrange(
                                    "p (s n) -> p s n", s=2)[:, :, 0:nq]
                                out3 = ets[hh][:, 0:2 * nq].rearrange(
                                    "p (s n) -> p s n", s=2)
                                nc.scalar.activation(out3, in3, EXP)
                                esl = ets[hh][:, 0:2 * nq].rearrange(
                                    "p (s n) -> p s n", s=2, n=nq)[:, :, 0:128]
                                nc.vector.scalar_tensor_tensor(
                                    esl, esl.bitcast(F32), 1.0,
                                    maskT01_sb[:, 128 * m0:128 * (m0 + 2)].rearrange(
                                        "p (s n) -> p s n", s=2),
                                    MULT_OP0, MULT)
                            for e in range(2):
                                kk = 4 * g4 + 2 * d2 + e
                                for hh in range(2):
                                    kv = (2 * j + hh) // 7
                                    nc.tensor.matmul(
                                        oTs[hh][:, 128 * g4:128 * g4 + nq],
                                        vsb[kk][:, 64 * kv:64 * (kv + 1)],
                                        ets[hh][:, nq * e:nq * (e + 1)],
                                        start=(kk == 0), stop=(kk == 15))
                    # normalize outT by the s-path denominators (transposed)
                    for hh in range(2):
                        pdT = ps_scores.tile([128, 1024], F32, tag="scores",
                                             name="pdT")
                        nc.tensor.transpose(pdT[0:97, 0:128], den4s[hh][:],
                                            id32_sb[:])
                        recb = small.tile([64, 512], F32, tag="recb", name="recb")
                        for t in range(4):
                            r1 = small.tile([1, 128], F32, tag="r1", name="r1")
                            nc.vector.reciprocal(r1[:],
                                                 pdT[32 * t:32 * t + 1, 0:128])
                            nc.gpsimd.partition_broadcast(
                                recb[:, 128 * t:128 * (t + 1)], r1[:])
                        nc.vector.tensor_tensor(
                            outT_sb[j][64 * hh:64 * hh + 64, :],
                            oTs[hh][:], recb[:], MULT)

            # ---- phase 3: output projection ----
            with tc.tile_pool(name="ph3s", bufs=2) as ph3s:
                for half in range(2):
                    wo_sb = wo_all[half]
                    for pc in range(4):
                        pf = ps_scores.tile([128, 1024], F32, tag="scores",
                                            name="pf")
                        for hc in range(7):
                            nc.tensor.matmul(
                                pf[:, 0:448],
                                outT_sb[hc][:, 128 * pc:128 * (pc + 1)],
                                wo_sb[hc][:], start=(hc == 0), stop=(hc == 6))
                        fo = ph3s.tile([128, 448], F32, tag="fo", name="fo")
                        nc.vector.tensor_copy(fo[:], pf[:, 0:448])
                        nc.sync.dma_start(
                            o_out[128 * pc:128 * (pc + 1),
                                  448 * half:448 * (half + 1)], fo[:])

    nc.compile()
    return nc


def _host_prep(hidden_states, cos, sin, Wq, bq, Wk, bk, Wv, bv, Wo):
    f32 = np.float32
    hidden_states = np.asarray(hidden_states, f32)
    cos = np.asarray(cos, f32)
    sin = np.asarray(sin, f32)
    Wq = np.asarray(Wq, f32); bq = np.asarray(bq, f32)
    Wk = np.asarray(Wk, f32); bk = np.asarray(bk, f32)
    Wv = np.asarray(Wv, f32); bv = np.asarray(bv, f32)
    Wo = np.asarray(Wo, f32)

    scale = f32(1.0 / np.sqrt(HD))
    WqT = np.ascontiguousarray((Wq * scale).T)
    bq_s = bq * scale
    bqT = np.ascontiguousarray(bq_s.reshape(7, 128).T)
    WkT = np.ascontiguousarray(Wk.T)
    bkT = bk[:, None].copy()
    WvT = np.ascontiguousarray(Wv.T)
    bvT = bv[:, None].copy()
    WoT = np.ascontiguousarray(Wo.T)
    ident = np.eye(128, dtype=f32)

    sgn = np.concatenate([-np.ones(32, f32), np.ones(32, f32)])[:, None]
    cosk = np.ascontiguousarray(np.tile(cos.T, (2, 1)))
    ssink = np.ascontiguousarray(np.tile(sin.T * sgn, (2, 1)))

    shared = dict(WqT=WqT, WkT=WkT, WvT=WvT, WoT=WoT,
                  bqT=bqT, bkT=bkT, bvT=bvT,
                  cosk=cosk, ssink=ssink, ident=ident)

    r = np.arange(128)
    in_maps = []
    hTs = [np.ascontiguousarray(hidden_states[b].T) for b in range(B)]
    for c in range(N_CORES):
        b, g = divmod(c, 4)
        pos = _positions(g)
        thr = 16 * g + 64 * (r // 16) + (r % 16)          # q row r -> max col
        maskS = np.where(np.arange(512)[None, :] <= thr[:, None],
                         f32(1), f32(0)).astype(f32)
        maskT4 = np.empty((128, 512), f32)
        for m in range(4):
            kcol = 128 * m + np.arange(128)
            maskT4[:, 128 * m:128 * (m + 1)] = np.where(
                kcol[:, None] <= thr[None, :], f32(1), f32(0))
        cosq = np.ascontiguousarray(np.tile(cos[pos].T, (2, 1)))
        ssinq = np.ascontiguousarray(np.tile(sin[pos].T * sgn, (2, 1)))
        m_ = dict(shared)
        m_.update(hiddenT=hTs[b], hQ=np.ascontiguousarray(hTs[b][:, pos]),
                  cosq=cosq, ssinq=ssinq, maskS=maskS, maskT4=maskT4)
        in_maps.append(m_)
    return in_maps


def _numpy_fallback(hidden_states, cos, sin, Wq, bq, Wk, bk, Wv, bv, Wo,
                    attention_mask):
    x = np.asarray(hidden_states, np.float32)
    G = NH // NKV
    scaling = 1.0 / np.sqrt(HD)
    q = (x @ np.asarray(Wq).T + bq).reshape(B, S, NH, HD).transpose(0, 2, 1, 3)
    k = (x @ np.asarray(Wk).T + bk).reshape(B, S, NKV, HD).transpose(0, 2, 1, 3)
    v = (x @ np.asarray(Wv).T + bv).reshape(B, S, NKV, HD).transpose(0, 2, 1, 3)
    c = np.asarray(cos)[None, None]
    s = np.asarray(sin)[None, None]

    def rot(t):
        return np.concatenate([-t[..., HD // 2:], t[..., :HD // 2]], -1)

    q = q * c + rot(q) * s
    k = k * c + rot(k) * s
    qg = q.reshape(B, NKV, G, S, HD)
    scores = np.einsum('bghsd,bgtd->bghst', qg, k) * scaling
    bias = np.where(np.asarray(attention_mask), 0.0, NEG).astype(np.float32)
    scores = scores + bias[:, :, None]
    scores = scores - scores.max(-1, keepdims=True)
    w = np.exp(scores)
    w = (w / w.sum(-1, keepdims=True)).astype(np.float32)
    out = np.einsum('bghst,bgtd->bghsd', w, v)
    out = out.reshape(B, NH, S, HD).transpose(0, 2, 1, 3).reshape(B, S, H)
    return (out @ np.asarray(Wo).T).astype(np.float32), w.reshape(B, NH, S, S)


def kernel(hidden_states, cos, sin, Wq, bq, Wk, bk, Wv, bv, Wo,
           attention_mask):
    am = np.asarray(attention_mask)
    causal = np.tril(np.ones((S, S), bool))
    if am.shape != (B, 1, S, S) or not np.array_equal(
            am, np.broadcast_to(causal[None, None], (B, 1, S, S))):
        return _numpy_fallback(hidden_states, cos, sin, Wq, bq, Wk, bk,
                               Wv, bv, Wo, attention_mask)

    import sys
    for p in ("/opt/trn_rl_repo", "/root/.axon_site/_ro/trn_rl_repo"):
        if p not in sys.path:
            sys.path.append(p)
    from concourse.bass_utils import run_bass_kernel_spmd

    if "nc" not in _CACHE:
        _CACHE["nc"] = _build()
    nc = _CACHE["nc"]

    in_maps = _host_prep(hidden_states, cos, sin, Wq, bq, Wk, bk, Wv, bv, Wo)
    res = run_bass_kernel_spmd(nc, in_maps, list(range(N_CORES)))

    attn_w = np.zeros((B, NH, S, S), np.float32)
    attn_o = np.zeros((B, S, H), np.float32)
    wv = attn_w.reshape(B, NH, 32, 4, 16, S)
    ov = attn_o.reshape(B, 32, 4, 16, H)
    for c in range(N_CORES):
        b, g = divmod(c, 4)
        r = res.results[c]
        wv[b, :, :, g, :, :] = r["w_out"].reshape(NH, 32, 16, S)
        ov[b, :, g, :, :] = r["o_out"].reshape(32, 16, H)
    return attn_o, attn_w


# revision 19
# speedup vs baseline: 1.0475x; 1.0475x over previous
"""GQA attention (14 q heads, 2 kv heads, RoPE, causal) on 8 Trainium2 cores.

Sharding: core = (batch b, group g); each core owns the 512 query positions
{16*(g+4a)+i : a<32, i<16} of batch b (16-wide blocks strided by 64), which
gives every core an identical causal work shape (uniform SPMD program) while
balancing the causal triangle. Outputs are disjoint per core; no collectives.

Per-core q-chunk t (128 packed rows) attends exactly k-tiles 0..4t+3 for every
core; the partially-masked diagonal window is the last 4 k-tiles, with the
bias pattern host-supplied per core. Scores are computed twice on PE: once as
s[q,k] (softmax + attn_weights output, denominator via ACT accum_out) and once
as sT[k,q] for A@V (denominator double-checked via a ones column in V).
"""

import numpy as np

B, S, H = 2, 2048, 896
NH, NKV, HD = 14, 2, 64
N_CORES = 8
NEG = -1e9

_CACHE = {}


def _positions(g):
    blocks = np.arange(32) * 4 + g
    return (blocks[:, None] * 16 + np.arange(16)[None, :]).reshape(-1)


def _build():
    import sys
    for p in ("/opt/trn_rl_repo", "/root/.axon_site/_ro/trn_rl_repo"):
        if p not in sys.path:
            sys.path.append(p)
    from contextlib import ExitStack
    import concourse.tile as tile
    from concourse import bacc, mybir

    dt = mybir.dt
    F32 = dt.float32
    F32R = dt.float32r
    EXP = mybir.ActivationFunctionType.Exp
    IDENT = mybir.ActivationFunctionType.Identity
    MULT = mybir.AluOpType.mult
    MULT_OP0 = mybir.AluOpType.mult
    ADD = mybir.AluOpType.add

    nc = bacc.Bacc("TRN2", target_bir_lowering=False, debug=False,
                   num_devices=N_CORES)

    def din(name, shape, dtype=F32R):
        return nc.dram_tensor(name, shape, dtype, kind="ExternalInput").ap()

    hiddenT = din("hiddenT", [H, S])
    hQ = din("hQ", [H, 512])
    WqT = din("WqT", [H, H])
    WkT = din("WkT", [H, 128])
    WvT = din("WvT", [H, 128])
    WoT = din("WoT", [H, H])
    bqT = din("bqT", [128, 7], F32)
    bkT = din("bkT", [128, 1], F32)
    bvT = din("bvT", [128, 1], F32)
    cosq = din("cosq", [128, 512])
    ssinq = din("ssinq", [128, 512])
    cosk = din("cosk", [128, S])
    ssink = din("ssink", [128, S])
    maskS = din("maskS", [128, 512], F32)
    maskT4 = din("maskT4", [128, 512], F32)
    # 0/1 variants (multiplicative masking on SBUF, off the PSUM path)
    ident = din("ident", [128, 128])

    w_out = nc.dram_tensor("w_out", [NH, 512, S], F32, kind="ExternalOutput").ap()
    o_out = nc.dram_tensor("o_out", [512, H], F32, kind="ExternalOutput").ap()

    with tile.TileContext(nc) as tc:
        with ExitStack() as es:
            consts = es.enter_context(tc.tile_pool(name="consts", bufs=1))
            persist = es.enter_context(tc.tile_pool(name="persist", bufs=1))
            ps_scores = es.enter_context(
                tc.tile_pool(name="scores", bufs=3, space="PSUM"))
            small = es.enter_context(tc.tile_pool(name="small", bufs=8))
            ph1sp = es.enter_context(tc.tile_pool(name="ph1sp", bufs=1))

            # ---- constants (persist whole kernel) ----
            maskS01_sb = consts.tile([128, 512], F32)
            nc.scalar.dma_start(maskS01_sb[:], maskS)
            maskT01_sb = consts.tile([128, 512], F32)
            nc.scalar.dma_start(maskT01_sb[:], maskT4)
            id_sb = consts.tile([128, 128], F32R)
            nc.scalar.dma_start(id_sb[:], ident)
            id32_sb = consts.tile([128, 128], F32)
            nc.scalar.dma_start(id32_sb[:], ident.bitcast(F32))
            bk_sb = consts.tile([128, 1], F32)
            nc.scalar.dma_start(bk_sb[:], bkT)
            bv_sb = consts.tile([128, 1], F32)
            nc.scalar.dma_start(bv_sb[:], bvT)
            bq_sb = consts.tile([128, 7], F32)
            nc.scalar.dma_start(bq_sb[:], bqT)

            # persistent phase-2 tensors (allocated as they are produced)
            qR = [persist.tile([128, 512], F32R, tag=f"qR{j}", name=f"qR{j}")
                  for j in range(7)]
            kt = ph1sp.tile([128, S], F32R)
            ktr = ph1sp.tile([128, S], F32R)
            vt = ph1sp.tile([128, S], F32R)

            # ---- phase 1a: projections (hidden + streamed weights) ----
            with tc.tile_pool(name="ph1a", bufs=1) as ph1a, \
                 tc.tile_pool(name="wload", bufs=3) as wload, \
                 tc.tile_pool(name="qtmp", bufs=3) as qtmp:
                hq = []
                for ic in range(7):
                    h_sb = ph1a.tile([128, 512], F32R, tag=f"hq{ic}", name=f"hq{ic}")
                    nc.scalar.dma_start(h_sb[:], hQ[128 * ic:128 * (ic + 1), :])
                    hq.append(h_sb)
                hid = []
                for ic in range(7):
                    h_sb = ph1a.tile([128, S], F32R, tag=f"hid{ic}", name=f"hid{ic}")
                    nc.sync.dma_start(h_sb[:], hiddenT[128 * ic:128 * (ic + 1), :])
                    hid.append(h_sb)
                cq_sb = ph1a.tile([128, 512], F32R)
                nc.scalar.dma_start(cq_sb[:], cosq)
                sq_sb = ph1a.tile([128, 512], F32R)
                nc.scalar.dma_start(sq_sb[:], ssinq)

                # q projections, streamed per out-chunk pair; rotate-half
                # comes from partition-shifted copies of the biased q.
                for ocg in range(4):
                    noc = 2 if ocg < 3 else 1
                    cols = slice(256 * ocg, 256 * ocg + 128 * noc)
                    pq = ps_scores.tile([128, 1024], F32, tag="scores", name="pq")
                    for ic in range(7):
                        wq_i = wload.tile([128, 256], F32R, tag="wqi", name="wq_i")
                        nc.scalar.dma_start(wq_i[:, 0:128 * noc],
                                          WqT[128 * ic:128 * (ic + 1), cols])
                        for e in range(noc):
                            nc.tensor.matmul(pq[:, 512 * e:512 * e + 512],
                                             wq_i[:, 128 * e:128 * (e + 1)],
                                             hq[ic][:],
                                             start=(ic == 0), stop=(ic == 6))
                    for e in range(noc):
                        oc = 2 * ocg + e
                        qt = qtmp.tile([128, 512], F32R, tag="qt", name="qt")
                        nc.scalar.activation(qt[:], pq[:, 512 * e:512 * e + 512],
                                             IDENT, bias=bq_sb[:, oc:oc + 1])
                        qtr = qtmp.tile([128, 512], F32R, tag="qtr", name="qtr")
                        for hb in range(2):
                            nc.vector.tensor_copy(
                                qtr[64 * hb:64 * hb + 32, :],
                                qt[64 * hb + 32:64 * hb + 64, :])
                            nc.vector.tensor_copy(
                                qtr[64 * hb + 32:64 * hb + 64, :],
                                qt[64 * hb:64 * hb + 32, :])
                        nc.vector.tensor_tensor(qt[:], qt[:].bitcast(F32),
                                                cq_sb[:].bitcast(F32), MULT)
                        nc.vector.tensor_tensor(qtr[:], qtr[:].bitcast(F32),
                                                sq_sb[:].bitcast(F32), MULT)
                        nc.vector.tensor_tensor(qR[oc][:], qt[:].bitcast(F32),
                                                qtr[:].bitcast(F32), ADD)

                # k / v projections (weights tiny: fully resident)
                wkv = []
                for nm, wsrc in (("wk", WkT), ("wv", WvT)):
                    tiles = []
                    for ic in range(7):
                        w_t = ph1a.tile([128, 128], F32R, tag=f"{nm}{ic}",
                                        name=f"{nm}{ic}")
                        nc.scalar.dma_start(w_t[:], wsrc[128 * ic:128 * (ic + 1), :])
                        tiles.append(w_t)
                    wkv.append(tiles)
                for w in range(4):
                    sl = slice(512 * w, 512 * (w + 1))
                    pk = ps_scores.tile([128, 1024], F32, tag="scores", name="pk")
                    pv = ps_scores.tile([128, 1024], F32, tag="scores", name="pv")
                    for ic in range(7):
                        nc.tensor.matmul(pk[:, 0:512], wkv[0][ic][:], hid[ic][:, sl],
                                         start=(ic == 0), stop=(ic == 6))
                        nc.tensor.matmul(pv[:, 0:512], wkv[1][ic][:], hid[ic][:, sl],
                                         start=(ic == 0), stop=(ic == 6))
                    nc.scalar.activation(kt[:, sl], pk[:, 0:512], IDENT, bias=bk_sb[:])
                    nc.scalar.activation(vt[:, sl], pv[:, 0:512], IDENT, bias=bv_sb[:])
                for hb in range(2):
                    nc.vector.tensor_copy(ktr[64 * hb:64 * hb + 32, :],
                                          kt[64 * hb + 32:64 * hb + 64, :])
                    nc.vector.tensor_copy(ktr[64 * hb + 32:64 * hb + 64, :],
                                          kt[64 * hb:64 * hb + 32, :])

            # ---- phase 1b: RoPE on k, kv-duplicated layouts, v transpose ----
            # opened only now so phase-1a gets the SBUF headroom
            persist2 = es.enter_context(tc.tile_pool(name="persist2", bufs=1))
            K00 = persist2.tile([128, S], F32R)
            K01 = persist2.tile([128, S], F32R)
            K11 = persist2.tile([128, S], F32R)
            vsb = [persist2.tile([128, 128], F32R, tag=f"v{k}", name=f"v{k}")
                   for k in range(16)]
            with tc.tile_pool(name="ph1b", bufs=1) as ph1b:
                ck_sb = ph1b.tile([128, S], F32R)
                nc.sync.dma_start(ck_sb[:], cosk)
                sk_sb = ph1b.tile([128, S], F32R)
                nc.sync.dma_start(sk_sb[:], ssink)
                nc.vector.tensor_tensor(kt[:], kt[:].bitcast(F32),
                                        ck_sb[:].bitcast(F32), MULT)
                nc.vector.tensor_tensor(ktr[:], ktr[:].bitcast(F32),
                                        sk_sb[:].bitcast(F32), MULT)
                nc.vector.tensor_tensor(K01[:], kt[:].bitcast(F32),
                                        ktr[:].bitcast(F32), ADD)
                nc.sync.dma_start(K00[0:64, :], K01[0:64, :])
                nc.sync.dma_start(K00[64:128, :], K01[0:64, :])
                nc.sync.dma_start(K11[0:64, :], K01[64:128, :])
                nc.sync.dma_start(K11[64:128, :], K01[64:128, :])
                for k in range(16):
                    pvT = ps_scores.tile([128, 1024], F32R, tag="scores", name="pvT")
                    nc.tensor.transpose(pvT[:, 0:128], vt[:, 128 * k:128 * (k + 1)],
                                        id_sb[:])
                    nc.vector.tensor_copy(vsb[k][:], pvT[:, 0:128])

            # ---- phase 2: attention, head-pairs interleaved ----
            outT_sb = [persist2.tile([128, 512], F32R, tag=f"oT{j}",
                                     name=f"oT{j}") for j in range(7)]
            KJ = [K00, K00, K00, K01, K11, K11, K11]
            ph3 = es.enter_context(tc.tile_pool(name="ph3", bufs=1))
            wo_all = []
            for half in range(2):
                row = []
                for hc in range(7):
                    w_t = ph3.tile([128, 448], F32R, tag=f"wo{half}_{hc}",
                                   name=f"wo{half}_{hc}")
                    nc.scalar.dma_start(
                        w_t[:], WoT[128 * hc:128 * (hc + 1),
                                    448 * half:448 * (half + 1)])
                    row.append(w_t)
                wo_all.append(row)
            with tc.tile_pool(name="wstage", bufs=3) as wstage, \
                 tc.tile_pool(name="estage", bufs=6) as estage, \
                 tc.tile_pool(name="ps_av", bufs=1, space="PSUM") as ps_av:
                for j in range(7):
                    kjf = KJ[j]
                    # --- s-path, both heads interleaved ---
                    # chunk-t denominator in column 32t so the PE transpose
                    # lands it at a 32-aligned partition
                    den4s = [small.tile([128, 97], F32, tag=f"den4_{hh}",
                                        name=f"den4_{hh}", bufs=2)
                             for hh in range(2)]
                    oTs = [ps_av.tile([64, 512], F32, tag=f"oT{hh}",
                                      name=f"oT{hh}") for hh in range(2)]
                    for t in range(4):
                        span = 512 * (t + 1)
                        wsb = [wstage.tile([128, 2048], F32, tag=f"w{hh}",
                                           name=f"w{hh}") for hh in range(2)]
                        # non-diagonal windows 0..t-1 (pairs of 512), then the
                        # diagonal window t alone; masking happens on SBUF.
                        pts = [[], []]
                        for tp in range((t + 1) // 2):
                            nwin = min(2, t - 2 * tp)
                            for hh in range(2):
                                pts[hh].append(
                                    (ps_scores.tile([128, 1024], F32,
                                                    tag="scores", name="pt"),
                                     nwin))
                            for e in range(nwin):
                                tau = 2 * tp + e
                                for hh in range(2):
                                    pi = 64 * hh
                                    nc.tensor.matmul(
                                        pts[hh][tp][0][:, 512 * e:512 * (e + 1)],
                                        qR[j][pi:pi + 64, 128 * t:128 * (t + 1)],
                                        kjf[pi:pi + 64, 512 * tau:512 * (tau + 1)],
                                        start=True, stop=True)
                        pds = []
                        for hh in range(2):
                            pi = 64 * hh
                            pd = ps_scores.tile([128, 1024], F32, tag="scores",
                                                name="pd")
                            nc.tensor.matmul(
                                pd[:, 0:512],
                                qR[j][pi:pi + 64, 128 * t:128 * (t + 1)],
                                kjf[pi:pi + 64, 512 * t:512 * (t + 1)],
                                start=True, stop=True)
                            pds.append(pd)
                        for hh in range(2):
                            dcol = 32 * t
                            dparts = []
                            for tp, (pt, nwin) in enumerate(pts[hh]):
                                acc = small.tile([128, 1], F32, tag="dpart",
                                                 name="dpart")
                                nc.scalar.activation(
                                    wsb[hh][:, 1024 * tp:1024 * tp + 512 * nwin],
                                    pt[:, 0:512 * nwin], EXP, accum_out=acc)
                                dparts.append(acc)
                            # diagonal: exp then 0/1-mask-mult with accum
                            nc.scalar.activation(wsb[hh][:, 512 * t:512 * (t + 1)],
                                                 pds[hh][:, 0:512], EXP)
                            nc.vector.scalar_tensor_tensor(
                                wsb[hh][:, 512 * t:512 * (t + 1)],
                                wsb[hh][:, 512 * t:512 * (t + 1)],
                                1.0, maskS01_sb[:], MULT_OP0, MULT,
                                accum_out=den4s[hh][:, dcol:dcol + 1])
                            for acc in dparts:
                                nc.vector.tensor_add(
                                    den4s[hh][:, dcol:dcol + 1],
                                    den4s[hh][:, dcol:dcol + 1], acc)
                        for hh in range(2):
                            dcol = 32 * t
                            rec = small.tile([128, 1], F32, tag="rec", name="rec")
                            nc.vector.reciprocal(rec[:],
                                                 den4s[hh][:, dcol:dcol + 1])
                            nc.vector.tensor_scalar_mul(wsb[hh][:, 0:span],
                                                        wsb[hh][:, 0:span], rec[:])
                            nc.sync.dma_start(
                                w_out[2 * j + hh, 128 * t:128 * (t + 1), 0:span],
                                wsb[hh][:, 0:span])

                    for g4 in range(4):
                        nq = 512 - 128 * g4
                        for d2 in range(2):
                            sts = []
                            ets = []
                            for hh in range(2):
                                sts.append(ps_scores.tile(
                                    [128, 1024], F32, tag="scores", name="st"))
                                ets.append(estage.tile(
                                    [128, 1024], F32R, tag="et", name="et"))
                            for e in range(2):
                                kk = 4 * g4 + 2 * d2 + e
                                for hh in range(2):
                                    pi = 64 * hh
                                    nc.tensor.matmul(
                                        sts[hh][:, 512 * e:512 * e + nq],
                                        kjf[pi:pi + 64, 128 * kk:128 * (kk + 1)],
                                        qR[j][pi:pi + 64, 128 * g4:512],
                                        start=True, stop=True)
                            for hh in range(2):
                                m0 = (4 * g4 + 2 * d2) % 4
                                in3 = sts[hh][:].rearrange(
                                    "p (s n) -> p s n", s=2)[:, :, 0:nq]
                                out3 = ets[hh][:, 0:2 * nq].rearrange(
                                    "p (s n) -> p s n", s=2)
                                nc.scalar.activation(out3, in3, EXP)
                                esl = ets[hh][:, 0:2 * nq].rearrange(
                                    "p (s n) -> p s n", s=2, n=nq)[:, :, 0:128]
                                nc.vector.scalar_tensor_tensor(
                                    esl, esl.bitcast(F32), 1.0,
                                    maskT01_sb[:, 128 * m0:128 * (m0 + 2)].rearrange(
                                        "p (s n) -> p s n", s=2),
                                    MULT_OP0, MULT)
                            for e in range(2):
                                kk = 4 * g4 + 2 * d2 + e
                                for hh in range(2):
                                    kv = (2 * j + hh) // 7
                                    nc.tensor.matmul(
                                        oTs[hh][:, 128 * g4:128 * g4 + nq],
                                        vsb[kk][:, 64 * kv:64 * (kv + 1)],
                                        ets[hh][:, nq * e:nq * (e + 1)],
                                        start=(kk == 0), stop=(kk == 15))
                    # normalize outT by the s-path denominators (transposed)
                    for hh in range(2):
                        pdT = ps_scores.tile([128, 1024], F32, tag="scores",
                                             name="pdT")
                        nc.tensor.transpose(pdT[0:97, 0:128], den4s[hh][:],
                                            id32_sb[:])
                        recb = small.tile([64, 512], F32, tag="recb", name="recb")
                        for t in range(4):
                            r1 = small.tile([1, 128], F32, tag="r1", name="r1")
                            nc.vector.reciprocal(r1[:],
                                                 pdT[32 * t:32 * t + 1, 0:128])
                            nc.gpsimd.partition_broadcast(
                                recb[:, 128 * t:128 * (t + 1)], r1[:])
                        nc.vector.tensor_tensor(
                            outT_sb[j][64 * hh:64 * hh + 64, :],
                            oTs[hh][:], recb[:], MULT)

            # ---- phase 3: output projection ----
            with tc.tile_pool(name="ph3s", bufs=2) as ph3s:
                for half in range(2):
                    wo_sb = wo_all[half]
                    for pc in range(4):
                        pf = ps_scores.tile([128, 1024], F32, tag="scores",
                                            name="pf")
                        for hc in range(7):
                            nc.tensor.matmul(
                                pf[:, 0:448],
                                outT_sb[hc][:, 128 * pc:128 * (pc + 1)],
                                wo_sb[hc][:], start=(hc == 0), stop=(hc == 6))
                        fo = ph3s.tile([128, 448], F32, tag="fo", name="fo")
                        nc.vector.tensor_copy(fo[:], pf[:, 0:448])
                        nc.sync.dma_start(
                            o_out[128 * pc:128 * (pc + 1),
                                  448 * half:448 * (half + 1)], fo[:])

    nc.compile()
    return nc


def _host_prep(hidden_states, cos, sin, Wq, bq, Wk, bk, Wv, bv, Wo):
    f32 = np.float32
    hidden_states = np.asarray(hidden_states, f32)
    cos = np.asarray(cos, f32)
    sin = np.asarray(sin, f32)
    Wq = np.asarray(Wq, f32); bq = np.asarray(bq, f32)
    Wk = np.asarray(Wk, f32); bk = np.asarray(bk, f32)
    Wv = np.asarray(Wv, f32); bv = np.asarray(bv, f32)
    Wo = np.asarray(Wo, f32)

    scale = f32(1.0 / np.sqrt(HD))
    WqT = np.ascontiguousarray((Wq * scale).T)
    bq_s = bq * scale
    bqT = np.ascontiguousarray(bq_s.reshape(7, 128).T)
    WkT = np.ascontiguousarray(Wk.T)
    bkT = bk[:, None].copy()
    WvT = np.ascontiguousarray(Wv.T)
    bvT = bv[:, None].copy()
    WoT = np.ascontiguousarray(Wo.T)
    ident = np.eye(128, dtype=f32)

    sgn = np.concatenate([-np.ones(32, f32), np.ones(32, f32)])[:, None]
    cosk = np.ascontiguousarray(np.tile(cos.T, (2, 1)))
    ssink = np.ascontiguousarray(np.tile(sin.T * sgn, (2, 1)))

    shared = dict(WqT=WqT, WkT=WkT, WvT=WvT, WoT=WoT,
                  bqT=bqT, bkT=bkT, bvT=bvT,
                  cosk=cosk, ssink=ssink, ident=ident)

    r = np.arange(128)
    in_maps = []
    hTs = [np.ascontiguousarray(hidden_states[b].T) for b in range(B)]
    for c in range(N_CORES):
        b, g = divmod(c, 4)
        pos = _positions(g)
        thr = 16 * g + 64 * (r // 16) + (r % 16)          # q row r -> max col
        maskS = np.where(np.arange(512)[None, :] <= thr[:, None],
                         f32(1), f32(0)).astype(f32)
        maskT4 = np.empty((128, 512), f32)
        for m in range(4):
            kcol = 128 * m + np.arange(128)
            maskT4[:, 128 * m:128 * (m + 1)] = np.where(
                kcol[:, None] <= thr[None, :], f32(1), f32(0))
        cosq = np.ascontiguousarray(np.tile(cos[pos].T, (2, 1)))
        ssinq = np.ascontiguousarray(np.tile(sin[pos].T * sgn, (2, 1)))
        m_ = dict(shared)
        m_.update(hiddenT=hTs[b], hQ=np.ascontiguousarray(hTs[b][:, pos]),
                  cosq=cosq, ssinq=ssinq, maskS=maskS, maskT4=maskT4)
        in_maps.append(m_)
    return in_maps


def _numpy_fallback(hidden_states, cos, sin, Wq, bq, Wk, bk, Wv, bv, Wo,
                    attention_mask):
    x = np.asarray(hidden_states, np.float32)
    G = NH // NKV
    scaling = 1.0 / np.sqrt(HD)
    q = (x @ np.asarray(Wq).T + bq).reshape(B, S, NH, HD).transpose(0, 2, 1, 3)
    k = (x @ np.asarray(Wk).T + bk).reshape(B, S, NKV, HD).transpose(0, 2, 1, 3)
    v = (x @ np.asarray(Wv).T + bv).reshape(B, S, NKV, HD).transpose(0, 2, 1, 3)
    c = np.asarray(cos)[None, None]
    s = np.asarray(sin)[None, None]

    def rot(t):
        return np.concatenate([-t[..., HD // 2:], t[..., :HD // 2]], -1)

    q = q * c + rot(q) * s
    k = k * c + rot(k) * s
    qg = q.reshape(B, NKV, G, S, HD)
    scores = np.einsum('bghsd,bgtd->bghst', qg, k) * scaling
    bias = np.where(np.asarray(attention_mask), 0.0, NEG).astype(np.float32)
    scores = scores + bias[:, :, None]
    scores = scores - scores.max(-1, keepdims=True)
    w = np.exp(scores)
    w = (w / w.sum(-1, keepdims=True)).astype(np.float32)
    out = np.einsum('bghst,bgtd->bghsd', w, v)
    out = out.reshape(B, NH, S, HD).transpose(0, 2, 1, 3).reshape(B, S, H)
    return (out @ np.asarray(Wo).T).astype(np.float32), w.reshape(B, NH, S, S)


def kernel(hidden_states, cos, sin, Wq, bq, Wk, bk, Wv, bv, Wo,
           attention_mask):
    am = np.asarray(attention_mask)
    causal = np.tril(np.ones((S, S), bool))
    if am.shape != (B, 1, S, S) or not np.array_equal(
            am, np.broadcast_to(causal[None, None], (B, 1, S, S))):
        return _numpy_fallback(hidden_states, cos, sin, Wq, bq, Wk, bk,
                               Wv, bv, Wo, attention_mask)

    import sys
    for p in ("/opt/trn_rl_repo", "/root/.axon_site/_ro/trn_rl_repo"):
        if p not in sys.path:
            sys.path.append(p)
    from concourse.bass_utils import run_bass_kernel_spmd

    if "nc" not in _CACHE:
        _CACHE["nc"] = _build()
    nc = _CACHE["nc"]

    in_maps = _host_prep(hidden_states, cos, sin, Wq, bq, Wk, bk, Wv, bv, Wo)
    res = run_bass_kernel_spmd(nc, in_maps, list(range(N_CORES)))

    attn_w = np.zeros((B, NH, S, S), np.float32)
    attn_o = np.zeros((B, S, H), np.float32)
    wv = attn_w.reshape(B, NH, 32, 4, 16, S)
    ov = attn_o.reshape(B, 32, 4, 16, H)
    for c in range(N_CORES):
        b, g = divmod(c, 4)
        r = res.results[c]
        wv[b, :, :, g, :, :] = r["w_out"].reshape(NH, 32, 16, S)
        ov[b, :, g, :, :] = r["o_out"].reshape(32, 16, H)
    return attn_o, attn_w
